# revision 38
# baseline (speedup 1.0000x reference)
"""Trainium2 SPMD kernel: StyleGAN2-style modulated conv (Conv2dWeightModulate).

Reference math (per batch sample b):
    w0        = weight * RC                       (equalized-lr scale)
    ws        = w0 * style[b][None,:,None,None]   (per-input-channel modulation)
    sigma_inv = rsqrt(sum_{I,K,K} ws^2 + eps)     (per-output-channel demodulation)
    out[b]    = conv2d(x[b], ws * sigma_inv, pad=1)

Because the modulation is a per-input-channel scale and conv is linear, this
factorizes into ops with a SHARED weight across the batch:
    out[b] = sigma_inv[b,:] * conv2d(x[b] * (style[b]*RC), weight)
    sigma_inv[b,o] = rsqrt(RC^2 * sum_{i,t} weight[o,i,t]^2 * style[b,i]^2 + eps)

Sharding: data-parallel over batch: 8 samples -> 8 NeuronCores, weight
replicated (the groups=b conv factorizes exactly across the batch).

Schedule (v4): all 8 PSUM banks hold the full [512 cout x 1024 px] output at
once as groups (oc, h) = (4 cout chunks x 2 pixel halves); the matmul stream
is chunk-major with all 8 groups interleaved so there is no PSUM wave
barrier. Matmuls run in bf16: the fp32r self-loading matmul's 190ns weight
load limits the cadence to ~245ns, bf16 loads halve that and reach the
213ns roofline. Weights are cast fp32->bf16 on DVE (GpSimd casts run at ~4
cycles/elem -- measured, do not move them back); DVE work is emitted in
consumption order (cast c, scale x_c, sigma-adds c-1) so the in-order queue
never parks behind a late DMA. The last input chunk runs oc/h-major so
groups retire every ~2us: oc=0 flushes UNSCALED to free its banks for the
sigma reduction, later groups flush through ACT with the sigma scale baked
in, and the final group's flush is split in quarters to pipeline ACT with
the output DMA.
"""

from contextlib import ExitStack

import numpy as np

import concourse.bass as bass
import concourse.tile as tile
from concourse import bacc, mybir
from concourse.bass_utils import run_bass_kernel_spmd

B = 8
CIN = 512
COUT = 512
KK = 3
H = 32
W = 32
PIX = H * W
NCH = 4  # channel chunks of 128
TAPS = KK * KK
RC = float(1.0 / np.sqrt(CIN * KK * KK))
EPS = 1e-8
F32 = mybir.dt.float32
F32R = mybir.dt.float32r
BF16 = mybir.dt.bfloat16
AF = mybir.ActivationFunctionType

# test.py toggles these; the grading harness just calls kernel().
TRACE = False
LAST_RESULTS = None


def _body(ctx, tc, x_d, st_d, wt_d, out_d):
    nc = tc.nc
    const = ctx.enter_context(tc.tile_pool(name="const", bufs=1))
    wpool = ctx.enter_context(tc.tile_pool(name="wpool", bufs=1))
    xpool = ctx.enter_context(tc.tile_pool(name="xpool", bufs=1))
    sqpool = ctx.enter_context(tc.tile_pool(name="sqpool", bufs=3))
    opool = ctx.enter_context(tc.tile_pool(name="opool", bufs=4))
    psum = ctx.enter_context(
        tc.tile_pool(name="psum", bufs=1, space=bass.MemorySpace.PSUM)
    )

    # --- all 8 PSUM banks: conv groups (oc chunk, pixel half) ---
    groups = [(oc, h) for oc in range(NCH) for h in range(2)]
    pc = {
        g: psum.tile([128, 512], F32, tag=f"g{g[0]}{g[1]}", name=f"pc{g[0]}{g[1]}")
        for g in groups
    }

    # --- PE pre-warm: ~3us of dummy matmuls so the HAM clock-gate is
    # already released (2.4 GHz) when the first real matmul issues.
    # Warm output aliases bank (0,0); conv's start=True resets it. ---
    # Full-size warms: tiny [1,1]x[1,128] warms leave the clock at the 1.2
    # GHz p-state and the first ~14 real matmuls then run at 427ns
    # (measured); only full-array load ramps the HAM clock to 2.4 GHz.
    # ~24 of these span the input-DMA window (~6.9 -> ~13.8us).
    warm_src = const.tile([128, 512], BF16, tag="warm_src")
    nc.gpsimd.memset(warm_src[:], 0.001)
    warm_w = const.tile([128, 128], BF16, tag="warm_w")
    nc.gpsimd.memset(warm_w[:], 0.001)
    warm_ps = pc[(0, 0)][:, :]
    for _ in range(24):
        nc.tensor.matmul(warm_ps, warm_w[:], warm_src[:], start=True, stop=True)

    # --- x chunk 0 + style are the critical path for the first matmul:
    # trigger them first, split x0 across the sync+gpsimd rings. ---
    # Chunk 0's padded picture is split into two tiles (rows 0-17 / 16-33,
    # two-row overlap re-fetched): dependencies are tile-granular, so with
    # one tile the first matmul waits for the whole of x0 + both scale ops.
    xs = []
    xst = []
    for c in range(NCH):
        if c == 0:
            xa = xpool.tile([128, 18, W + 2], BF16, tag="xs0a", name="xs0a")
            xb = xpool.tile([128, 18, W + 2], BF16, tag="xs0b", name="xs0b")
            xta = xpool.tile([128, 17, W], F32, tag="xst0a", name="xst0a")
            xtb = xpool.tile([128, 17, W], F32, tag="xst0b", name="xst0b")
            xs.append((xa, xb))
            xst.append((xta, xtb))
        else:
            xc = xpool.tile([128, H + 2, W + 2], BF16, tag=f"xs{c}", name=f"xs{c}")
            xt = xpool.tile([128, H, W], F32, tag=f"xst{c}", name=f"xst{c}")
            xs.append(xc)
            xst.append(xt)

    # style goes first: its 128 16-byte descriptors complete ~0.6us after
    # entering the queues, but queued behind x0 they'd finish ~3us late and
    # st_rc gates the x-scale on the first-matmul critical path.
    st = const.tile([128, NCH], F32, tag="st")
    x0 = x_d[0].rearrange("p (h w) -> p h w", h=H)
    nc.sync.dma_start(st[:], st_d[:])
    nc.sync.dma_start(xst[0][0][:], x0[:, 0:17, :])

    st_rc = const.tile([128, NCH], F32, tag="st_rc")
    nc.vector.tensor_scalar_mul(st_rc[:], st[:], RC)
    st2 = const.tile([128, NCH], BF16, tag="st2")
    nc.vector.tensor_mul(st2[:], st[:], st[:])

    # padded-picture borders on GpSimd (it has nothing else to do)
    xa, xb = xs[0]
    nc.gpsimd.memset(xa[:, 0, :], 0.0)
    nc.gpsimd.memset(xa[:, 1:18, 0], 0.0)
    nc.gpsimd.memset(xa[:, 1:18, W + 1], 0.0)
    nc.gpsimd.memset(xb[:, 17, :], 0.0)
    nc.gpsimd.memset(xb[:, 0:17, 0], 0.0)
    nc.gpsimd.memset(xb[:, 0:17, W + 1], 0.0)
    for c in range(1, NCH):
        nc.gpsimd.memset(xs[c][:, 0, :], 0.0)
        nc.gpsimd.memset(xs[c][:, H + 1, :], 0.0)
        nc.gpsimd.memset(xs[c][:, 1 : H + 1, 0], 0.0)
        nc.gpsimd.memset(xs[c][:, 1 : H + 1, W + 1], 0.0)

    # --- weights: per-tap triggers for chunk 0 so tap t streams just ahead
    # of the PE; later chunks one trigger each; x1-3 behind the weight
    # chunk that precedes their use. All on the scalar ring. ---
    wt = [
        wpool.tile([128, TAPS, COUT], F32, tag=f"wt{c}", name=f"wt{c}")
        for c in range(NCH)
    ]
    wt16 = [
        wpool.tile([128, TAPS, COUT], BF16, tag=f"wt16_{c}", name=f"wt16_{c}")
        for c in range(NCH)
    ]
    nc.gpsimd.dma_start(xst[0][1][:], x0[:, 15:H, :])
    for t in range(TAPS):
        nc.scalar.dma_start(wt[0][:, t], wt_d[:, 0, t])

    # Per-chunk sum over taps of squared weights (ACT squares, DVE adds):
    # cuts the PE cost of the sigma reduction from 36 matmuls to 4.
    w2s = {}

    def sig_squares(c):
        parts = []
        for t in range(TAPS):
            w2 = sqpool.tile([128, COUT], BF16, tag=f"w2_{t % 3}", name="w2")
            nc.scalar.activation(w2[:], wt[c][:, t], AF.Square)
            parts.append(w2)
            if t == 1:
                acc = sqpool.tile([128, COUT], BF16, tag=f"w2s{c}", name="w2s")
                nc.vector.tensor_add(acc[:], parts[0][:], parts[1][:])
            elif t > 1:
                nc.vector.tensor_add(acc[:], acc[:], parts[-1][:])
        w2s[c] = acc

    # DVE in consumption order: cast chunk c, scale x_c, sigma-adds c-1.
    # ACT squares are emitted here too (in-order per engine, c-major).
    def casts(c, lo=0, hi=TAPS):
        for t in range(lo, hi):
            nc.vector.tensor_copy(wt16[c][:, t], wt[c][:, t])

    def xscale(c):
        nc.vector.tensor_scalar_mul(
            xs[c][:, 1 : H + 1, 1 : W + 1], xst[c][:], st_rc[:, c : c + 1]
        )

    # head: tap 0 is cast on ACT (in parallel with DVE's x-scale; both are
    # on the first-matmul critical path), DVE scales the rows the h=0
    # groups touch (arriving on the sync ring alone), then backfills.
    nc.scalar.activation(wt16[0][:, 0], wt[0][:, 0], AF.Copy)
    nc.vector.tensor_scalar_mul(xa[:, 1:18, 1 : W + 1], xst[0][0][:], st_rc[:, 0:1])
    nc.vector.tensor_scalar_mul(xb[:, 0:17, 1 : W + 1], xst[0][1][:], st_rc[:, 0:1])
    casts(0, 1, 3)
    # Gate the chunk-1..3 DMAs (8.1 MB) behind the chunk-0 tap-2 cast: a
    # 1-element DVE copy reading wt16[0] tap 2 (a real dependency the
    # scheduler can't hoist) writes each gated tile, and the dma_start
    # emitted after it picks up the WAW dep. Keeps 8.1 MB of descriptors
    # out of the 16 DMA queues while x0 and the chunk-0 taps -- the
    # first-matmul critical path -- are in flight.
    for c in range(1, NCH):
        nc.vector.tensor_copy(wt[c][0:1, 0:1, 0:1], wt16[0][0:1, 2:3, 0:1])
        nc.scalar.dma_start(wt[c][:], wt_d[:, c])
        nc.vector.tensor_copy(xst[c][0:1, 0:1, 0:1], wt16[0][0:1, 2:3, 0:1])
        nc.scalar.dma_start(xst[c][:], x_d[c].rearrange("p (h w) -> p h w", h=H))
    casts(0, 3, TAPS)
    sig_squares(0)
    casts(1)
    xscale(1)
    sig_squares(1)
    casts(2)
    xscale(2)
    sig_squares(2)
    casts(3)
    xscale(3)
    sig_squares(3)

    eps_b = const.tile([1, 1], F32, tag="eps_b")
    nc.gpsimd.memset(eps_b[:], EPS)
    ones_t = const.tile([1, 1], F32, tag="ones_t")
    nc.gpsimd.memset(ones_t[:], 1.0)
    sig_sq = const.tile([1, COUT], F32, tag="sig_sq")
    sig_sd = const.tile([128, NCH], F32, tag="sig_sd")
    sig_t = const.tile([128, NCH], F32, tag="sig_t")

    # preload the Sqrt activation table while ACT is otherwise idle
    nc.scalar.activation(sig_sq[0:1, 0:1], eps_b[:], AF.Sqrt)

    # sigma PSUM space aliases group oc=0's banks, free after its early flush
    sig_ps = pc[(0, 0)][0:1, :]
    sig_tp = pc[(0, 1)][:, 0:NCH]

    out_ring = {0: nc.gpsimd, 1: nc.sync, 2: nc.gpsimd, 3: nc.sync}

    def flush_scaled(g):
        oc, h = g
        ob = opool.tile([128, 512], F32, tag=f"ob{h}", name=f"ob{h}")
        nc.scalar.activation(ob[:], pc[g][:], AF.Copy, scale=sig_t[:, oc : oc + 1])
        out_ring[oc].dma_start(out_d[oc, :, h * 512 : (h + 1) * 512], ob[:])

    def conv_mm(g, c, t, start, stop):
        oc, h = g
        dy, dx = t // 3, t % 3
        if c == 0:
            rhs = xs[0][h][:, dy : dy + 16, dx : dx + W]
        else:
            rhs = xs[c][:, dy + h * 16 : dy + h * 16 + 16, dx : dx + W]
        nc.tensor.matmul(
            pc[g][:],
            wt16[c][:, t, oc * 128 : (oc + 1) * 128],
            rhs,
            start=start,
            stop=stop,
        )

    # --- conv stream: chunks 0-2 with oc innermost (all 8 groups in
    # flight; h=0 groups first so chunk 0 starts before the second half of
    # x0 is scaled); chunk 3 oc/h-major so groups retire every ~2us. ---
    h_major = [(oc, h) for h in range(2) for oc in range(NCH)]
    for c in range(3):
        first = c == 0
        for t in range(TAPS):
            for g in h_major:
                conv_mm(g, c, t, first and t == 0, False)

    ou0 = [
        opool.tile([128, 512], F32, tag=f"ou0{h}", name=f"ou0{h}") for h in range(2)
    ]

    def sig_mms():
        for c in range(NCH):
            nc.tensor.matmul(
                sig_ps,
                st2[:, c : c + 1],
                w2s[c][:],
                start=(c == 0),
                stop=(c == NCH - 1),
            )

    def sig_transposes():
        # sqrt result [1,512] -> PE-transpose -> [128,4]
        for oc in range(NCH):
            nc.tensor.transpose(
                sig_tp[:, oc : oc + 1],
                sig_sq[0:1, oc * 128 : (oc + 1) * 128],
                ones_t[:],
            )

    # chunk 3, group-retiring order with the sigma pipeline woven in:
    #   PE : [oc0 h0] [oc0 h1] [sig mms] [oc1 h0] [transposes] [oc1 h1] ...
    #   ACT: ........ [flush oc0h0 raw] [flush oc0h1 raw] [sqrt] [flush oc1 scaled] ...
    #   DVE: ................................... [copy+recip] ... [scale ou0] ...
    for oc in range(NCH):
        for h in range(2):
            for t in range(TAPS):
                conv_mm((oc, h), 3, t, False, t == TAPS - 1)
            if oc == 0:
                # unscaled flush frees bank (0,h) for the sigma reduction
                nc.scalar.activation(ou0[h][:], pc[(0, h)][:], AF.Copy)
                if h == 1:
                    sig_mms()
            elif oc == 1:
                if h == 0:
                    nc.scalar.activation(
                        sig_sq[:], sig_ps, AF.Sqrt, bias=eps_b[:], scale=RC * RC
                    )
                    sig_transposes()
                else:
                    nc.vector.tensor_copy(sig_sd[:], sig_tp[:, 0:NCH])
                    nc.vector.reciprocal(sig_t[:], sig_sd[:])
                    flush_scaled((1, 0))
            elif oc == 2:
                if h == 0:
                    flush_scaled((1, 1))
                    # group 0 was flushed unscaled; apply sigma on DVE
                    for hh in range(2):
                        nc.vector.tensor_scalar_mul(
                            ou0[hh][:], ou0[hh][:], sig_t[:, 0:1]
                        )
                        out_ring[2 * hh].dma_start(
                            out_d[0, :, hh * 512 : (hh + 1) * 512], ou0[hh][:]
                        )
                else:
                    flush_scaled((2, 0))
            else:
                if h == 0:
                    flush_scaled((2, 1))
                    flush_scaled((3, 0))

    # final group: half-split flush; the two DMA triggers (~600ns DIRECT2D
    # each) run on different rings so descriptor generation overlaps
    obf = opool.tile([128, 512], F32, tag="obf", name="obf")
    rings = [nc.sync, nc.sync]
    for q in range(2):
        sl = slice(q * 256, (q + 1) * 256)
        nc.scalar.activation(obf[:, sl], pc[(3, 1)][:, sl], AF.Copy, scale=sig_t[:, 3:4])
        rings[q].dma_start(out_d[3, :, 512 + q * 256 : 512 + (q + 1) * 256], obf[:, sl])


_CACHE = None


def _get_compiled():
    global _CACHE
    if _CACHE is None:
        nc = bacc.Bacc(
            "TRN2", target_bir_lowering=False, debug=False, num_devices=B
        )
        x_d = nc.dram_tensor("x", [NCH, 128, PIX], F32, kind="ExternalInput").ap()
        st_d = nc.dram_tensor("style", [128, NCH], F32, kind="ExternalInput").ap()
        wt_d = nc.dram_tensor(
            "wt", [128, NCH, TAPS, COUT], F32, kind="ExternalInput"
        ).ap()
        out_d = nc.dram_tensor("out", [NCH, 128, PIX], F32, kind="ExternalOutput").ap()
        with tile.TileContext(nc) as tc, ExitStack() as ctx:
            _body(ctx, tc, x_d, st_d, wt_d, out_d)
        nc.compile()
        _CACHE = nc
    return _CACHE


def kernel(x, style, weight):
    """x: (8,512,32,32) f32, style: (8,512) f32, weight: (512,512,3,3) f32
    -> (8,512,32,32) f32"""
    global LAST_RESULTS
    x = np.ascontiguousarray(np.asarray(x, dtype=np.float32))
    style = np.asarray(style, dtype=np.float32)
    weight = np.asarray(weight, dtype=np.float32)

    # Host-side layout only (no arithmetic): lhsT weight layout
    # wt[i_lo, c, t, o] = weight[o, c*128 + i_lo, t//3, t%3]
    wt = np.ascontiguousarray(
        weight.reshape(COUT, NCH, 128, TAPS).transpose(2, 1, 3, 0)
    )
    in_maps = []
    for b in range(B):
        in_maps.append(
            {
                "x": x[b].reshape(NCH, 128, PIX),
                "style": np.ascontiguousarray(style[b].reshape(NCH, 128).T),
                "wt": wt,
            }
        )

    nc = _get_compiled()
    res = run_bass_kernel_spmd(nc, in_maps, list(range(B)), trace=TRACE)
    LAST_RESULTS = res
    out = np.empty((B, COUT, H, W), dtype=np.float32)
    for b in range(B):
        out[b] = res.results[b]["out"].reshape(COUT, H, W)
    return out


# revision 39
# speedup vs baseline: 1.0115x; 1.0115x over previous
"""Trainium2 SPMD kernel: StyleGAN2-style modulated conv (Conv2dWeightModulate).

Reference math (per batch sample b):
    w0        = weight * RC                       (equalized-lr scale)
    ws        = w0 * style[b][None,:,None,None]   (per-input-channel modulation)
    sigma_inv = rsqrt(sum_{I,K,K} ws^2 + eps)     (per-output-channel demodulation)
    out[b]    = conv2d(x[b], ws * sigma_inv, pad=1)

Because the modulation is a per-input-channel scale and conv is linear, this
factorizes into ops with a SHARED weight across the batch:
    out[b] = sigma_inv[b,:] * conv2d(x[b] * (style[b]*RC), weight)
    sigma_inv[b,o] = rsqrt(RC^2 * sum_{i,t} weight[o,i,t]^2 * style[b,i]^2 + eps)

Sharding: data-parallel over batch: 8 samples -> 8 NeuronCores, weight
replicated (the groups=b conv factorizes exactly across the batch).

Schedule (v4): all 8 PSUM banks hold the full [512 cout x 1024 px] output at
once as groups (oc, h) = (4 cout chunks x 2 pixel halves); the matmul stream
is chunk-major with all 8 groups interleaved so there is no PSUM wave
barrier. Matmuls run in bf16: the fp32r self-loading matmul's 190ns weight
load limits the cadence to ~245ns, bf16 loads halve that and reach the
213ns roofline. Weights are cast fp32->bf16 on DVE (GpSimd casts run at ~4
cycles/elem -- measured, do not move them back); DVE work is emitted in
consumption order (cast c, scale x_c, sigma-adds c-1) so the in-order queue
never parks behind a late DMA. The last input chunk runs oc/h-major so
groups retire every ~2us: oc=0 flushes UNSCALED to free its banks for the
sigma reduction, later groups flush through ACT with the sigma scale baked
in, and the final group's flush is split in quarters to pipeline ACT with
the output DMA.
"""

from contextlib import ExitStack

import numpy as np

import concourse.bass as bass
import concourse.tile as tile
from concourse import bacc, mybir
from concourse.bass_utils import run_bass_kernel_spmd

B = 8
CIN = 512
COUT = 512
KK = 3
H = 32
W = 32
PIX = H * W
NCH = 4  # channel chunks of 128
TAPS = KK * KK
RC = float(1.0 / np.sqrt(CIN * KK * KK))
EPS = 1e-8
F32 = mybir.dt.float32
F32R = mybir.dt.float32r
BF16 = mybir.dt.bfloat16
AF = mybir.ActivationFunctionType

# test.py toggles these; the grading harness just calls kernel().
TRACE = False
LAST_RESULTS = None


def _body(ctx, tc, x_d, st_d, wt_d, out_d):
    nc = tc.nc
    const = ctx.enter_context(tc.tile_pool(name="const", bufs=1))
    wpool = ctx.enter_context(tc.tile_pool(name="wpool", bufs=1))
    xpool = ctx.enter_context(tc.tile_pool(name="xpool", bufs=1))
    sqpool = ctx.enter_context(tc.tile_pool(name="sqpool", bufs=3))
    opool = ctx.enter_context(tc.tile_pool(name="opool", bufs=4))
    psum = ctx.enter_context(
        tc.tile_pool(name="psum", bufs=1, space=bass.MemorySpace.PSUM)
    )

    # --- all 8 PSUM banks: conv groups (oc chunk, pixel half) ---
    groups = [(oc, h) for oc in range(NCH) for h in range(2)]
    pc = {
        g: psum.tile([128, 512], F32, tag=f"g{g[0]}{g[1]}", name=f"pc{g[0]}{g[1]}")
        for g in groups
    }

    # --- PE pre-warm: ~3us of dummy matmuls so the HAM clock-gate is
    # already released (2.4 GHz) when the first real matmul issues.
    # Warm output aliases bank (0,0); conv's start=True resets it. ---
    # Full-size warms: tiny [1,1]x[1,128] warms leave the clock at the 1.2
    # GHz p-state and the first ~14 real matmuls then run at 427ns
    # (measured); only full-array load ramps the HAM clock to 2.4 GHz.
    # ~24 of these span the input-DMA window (~6.9 -> ~13.8us).
    warm_src = const.tile([128, 512], BF16, tag="warm_src")
    nc.vector.memset(warm_src[:], 0.001)
    warm_w = const.tile([128, 128], BF16, tag="warm_w")
    nc.vector.memset(warm_w[:], 0.001)
    warm_ps = pc[(0, 0)][:, :]
    for _ in range(14):
        nc.tensor.matmul(warm_ps, warm_w[:], warm_src[:], start=True, stop=True)

    # --- x chunk 0 + style are the critical path for the first matmul:
    # trigger them first, split x0 across the sync+gpsimd rings. ---
    # Chunk 0's padded picture is split into two tiles (rows 0-17 / 16-33,
    # two-row overlap re-fetched): dependencies are tile-granular, so with
    # one tile the first matmul waits for the whole of x0 + both scale ops.
    xs = []
    xst = []
    for c in range(NCH):
        if c == 0:
            xa = xpool.tile([128, 18, W + 2], BF16, tag="xs0a", name="xs0a")
            xb = xpool.tile([128, 18, W + 2], BF16, tag="xs0b", name="xs0b")
            xta = xpool.tile([128, 17, W], F32, tag="xst0a", name="xst0a")
            xtb = xpool.tile([128, 17, W], F32, tag="xst0b", name="xst0b")
            xs.append((xa, xb))
            xst.append((xta, xtb))
        else:
            xc = xpool.tile([128, H + 2, W + 2], BF16, tag=f"xs{c}", name=f"xs{c}")
            xt = xpool.tile([128, H, W], F32, tag=f"xst{c}", name=f"xst{c}")
            xs.append(xc)
            xst.append(xt)

    # style goes first: its 128 16-byte descriptors complete ~0.6us after
    # entering the queues, but queued behind x0 they'd finish ~3us late and
    # st_rc gates the x-scale on the first-matmul critical path.
    st = const.tile([128, NCH], F32, tag="st")
    x0 = x_d[0].rearrange("p (h w) -> p h w", h=H)
    nc.sync.dma_start(st[:], st_d[:])
    nc.sync.dma_start(xst[0][0][:], x0[:, 0:17, :])

    st_rc = const.tile([128, NCH], F32, tag="st_rc")
    nc.vector.tensor_scalar_mul(st_rc[:], st[:], RC)
    st2 = const.tile([128, NCH], BF16, tag="st2")
    nc.vector.tensor_mul(st2[:], st[:], st[:])

    # padded-picture borders on GpSimd (it has nothing else to do)
    xa, xb = xs[0]
    nc.gpsimd.memset(xa[:, 0, :], 0.0)
    nc.gpsimd.memset(xa[:, 1:18, 0], 0.0)
    nc.gpsimd.memset(xa[:, 1:18, W + 1], 0.0)
    nc.gpsimd.memset(xb[:, 17, :], 0.0)
    nc.gpsimd.memset(xb[:, 0:17, 0], 0.0)
    nc.gpsimd.memset(xb[:, 0:17, W + 1], 0.0)
    for c in range(1, NCH):
        nc.gpsimd.memset(xs[c][:, 0, :], 0.0)
        nc.gpsimd.memset(xs[c][:, H + 1, :], 0.0)
        nc.gpsimd.memset(xs[c][:, 1 : H + 1, 0], 0.0)
        nc.gpsimd.memset(xs[c][:, 1 : H + 1, W + 1], 0.0)

    # --- weights: per-tap triggers for chunk 0 so tap t streams just ahead
    # of the PE; later chunks one trigger each; x1-3 behind the weight
    # chunk that precedes their use. All on the scalar ring. ---
    wt = [
        wpool.tile([128, TAPS, COUT], F32, tag=f"wt{c}", name=f"wt{c}")
        for c in range(NCH)
    ]
    wt16 = [
        wpool.tile([128, TAPS, COUT], BF16, tag=f"wt16_{c}", name=f"wt16_{c}")
        for c in range(NCH)
    ]
    nc.gpsimd.dma_start(xst[0][1][:], x0[:, 15:H, :])
    for t in range(TAPS):
        nc.scalar.dma_start(wt[0][:, t], wt_d[:, 0, t])

    # Per-chunk sum over taps of squared weights (ACT squares, DVE adds):
    # cuts the PE cost of the sigma reduction from 36 matmuls to 4.
    w2s = {}

    def sig_squares(c):
        parts = []
        for t in range(TAPS):
            w2 = sqpool.tile([128, COUT], BF16, tag=f"w2_{t % 3}", name="w2")
            nc.scalar.activation(w2[:], wt[c][:, t], AF.Square)
            parts.append(w2)
            if t == 1:
                acc = sqpool.tile([128, COUT], BF16, tag=f"w2s{c}", name="w2s")
                nc.vector.tensor_add(acc[:], parts[0][:], parts[1][:])
            elif t > 1:
                nc.vector.tensor_add(acc[:], acc[:], parts[-1][:])
        w2s[c] = acc

    # DVE in consumption order: cast chunk c, scale x_c, sigma-adds c-1.
    # ACT squares are emitted here too (in-order per engine, c-major).
    def casts(c, lo=0, hi=TAPS):
        for t in range(lo, hi):
            nc.vector.tensor_copy(wt16[c][:, t], wt[c][:, t])

    def xscale(c):
        nc.vector.tensor_scalar_mul(
            xs[c][:, 1 : H + 1, 1 : W + 1], xst[c][:], st_rc[:, c : c + 1]
        )

    # head: tap 0 is cast on ACT (in parallel with DVE's x-scale; both are
    # on the first-matmul critical path), DVE scales the rows the h=0
    # groups touch (arriving on the sync ring alone), then backfills.
    nc.scalar.activation(wt16[0][:, 0], wt[0][:, 0], AF.Copy)
    nc.vector.tensor_scalar_mul(xa[:, 1:18, 1 : W + 1], xst[0][0][:], st_rc[:, 0:1])
    nc.vector.tensor_scalar_mul(xb[:, 0:17, 1 : W + 1], xst[0][1][:], st_rc[:, 0:1])
    casts(0, 1, 3)
    # Gate the chunk-1..3 DMAs (8.1 MB) behind the chunk-0 tap-2 cast: a
    # 1-element DVE copy reading wt16[0] tap 2 (a real dependency the
    # scheduler can't hoist) writes each gated tile, and the dma_start
    # emitted after it picks up the WAW dep. Keeps 8.1 MB of descriptors
    # out of the 16 DMA queues while x0 and the chunk-0 taps -- the
    # first-matmul critical path -- are in flight.
    for c in range(1, NCH):
        nc.vector.tensor_copy(wt[c][0:1, 0:1, 0:1], wt16[0][0:1, 2:3, 0:1])
        nc.scalar.dma_start(wt[c][:], wt_d[:, c])
        nc.vector.tensor_copy(xst[c][0:1, 0:1, 0:1], wt16[0][0:1, 2:3, 0:1])
        nc.scalar.dma_start(xst[c][:], x_d[c].rearrange("p (h w) -> p h w", h=H))
    casts(0, 3, TAPS)
    sig_squares(0)
    casts(1)
    xscale(1)
    sig_squares(1)
    casts(2)
    xscale(2)
    sig_squares(2)
    casts(3)
    xscale(3)
    sig_squares(3)

    eps_b = const.tile([1, 1], F32, tag="eps_b")
    nc.gpsimd.memset(eps_b[:], EPS)
    ones_t = const.tile([1, 1], F32, tag="ones_t")
    nc.gpsimd.memset(ones_t[:], 1.0)
    sig_sq = const.tile([1, COUT], F32, tag="sig_sq")
    sig_sd = const.tile([128, NCH], F32, tag="sig_sd")
    sig_t = const.tile([128, NCH], F32, tag="sig_t")

    # preload the Sqrt activation table while ACT is otherwise idle
    nc.scalar.activation(sig_sq[0:1, 0:1], eps_b[:], AF.Sqrt)

    # sigma PSUM space aliases group oc=0's banks, free after its early flush
    sig_ps = pc[(0, 0)][0:1, :]
    sig_tp = pc[(0, 1)][:, 0:NCH]

    out_ring = {0: nc.gpsimd, 1: nc.sync, 2: nc.gpsimd, 3: nc.sync}

    def flush_scaled(g):
        oc, h = g
        ob = opool.tile([128, 512], F32, tag=f"ob{h}", name=f"ob{h}")
        nc.scalar.activation(ob[:], pc[g][:], AF.Copy, scale=sig_t[:, oc : oc + 1])
        out_ring[oc].dma_start(out_d[oc, :, h * 512 : (h + 1) * 512], ob[:])

    def conv_mm(g, c, t, start, stop):
        oc, h = g
        dy, dx = t // 3, t % 3
        if c == 0:
            rhs = xs[0][h][:, dy : dy + 16, dx : dx + W]
        else:
            rhs = xs[c][:, dy + h * 16 : dy + h * 16 + 16, dx : dx + W]
        nc.tensor.matmul(
            pc[g][:],
            wt16[c][:, t, oc * 128 : (oc + 1) * 128],
            rhs,
            start=start,
            stop=stop,
        )

    # --- conv stream: chunks 0-2 with oc innermost (all 8 groups in
    # flight; h=0 groups first so chunk 0 starts before the second half of
    # x0 is scaled); chunk 3 oc/h-major so groups retire every ~2us. ---
    h_major = [(oc, h) for h in range(2) for oc in range(NCH)]
    for c in range(3):
        first = c == 0
        for t in range(TAPS):
            for g in h_major:
                conv_mm(g, c, t, first and t == 0, False)

    ou0 = [
        opool.tile([128, 512], F32, tag=f"ou0{h}", name=f"ou0{h}") for h in range(2)
    ]

    def sig_mms():
        for c in range(NCH):
            nc.tensor.matmul(
                sig_ps,
                st2[:, c : c + 1],
                w2s[c][:],
                start=(c == 0),
                stop=(c == NCH - 1),
            )

    def sig_transposes():
        # sqrt result [1,512] -> PE-transpose -> [128,4]
        for oc in range(NCH):
            nc.tensor.transpose(
                sig_tp[:, oc : oc + 1],
                sig_sq[0:1, oc * 128 : (oc + 1) * 128],
                ones_t[:],
            )

    # chunk 3, group-retiring order with the sigma pipeline woven in:
    #   PE : [oc0 h0] [oc0 h1] [sig mms] [oc1 h0] [transposes] [oc1 h1] ...
    #   ACT: ........ [flush oc0h0 raw] [flush oc0h1 raw] [sqrt] [flush oc1 scaled] ...
    #   DVE: ................................... [copy+recip] ... [scale ou0] ...
    for oc in range(NCH):
        for h in range(2):
            for t in range(TAPS):
                conv_mm((oc, h), 3, t, False, t == TAPS - 1)
            if oc == 0:
                # unscaled flush frees bank (0,h) for the sigma reduction
                nc.scalar.activation(ou0[h][:], pc[(0, h)][:], AF.Copy)
                if h == 1:
                    sig_mms()
            elif oc == 1:
                if h == 0:
                    nc.scalar.activation(
                        sig_sq[:], sig_ps, AF.Sqrt, bias=eps_b[:], scale=RC * RC
                    )
                    sig_transposes()
                else:
                    nc.vector.tensor_copy(sig_sd[:], sig_tp[:, 0:NCH])
                    nc.vector.reciprocal(sig_t[:], sig_sd[:])
                    flush_scaled((1, 0))
            elif oc == 2:
                if h == 0:
                    flush_scaled((1, 1))
                    # group 0 was flushed unscaled; apply sigma on DVE
                    for hh in range(2):
                        nc.vector.tensor_scalar_mul(
                            ou0[hh][:], ou0[hh][:], sig_t[:, 0:1]
                        )
                        out_ring[2 * hh].dma_start(
                            out_d[0, :, hh * 512 : (hh + 1) * 512], ou0[hh][:]
                        )
                else:
                    flush_scaled((2, 0))
            else:
                if h == 0:
                    flush_scaled((2, 1))
                    flush_scaled((3, 0))

    # final group: half-split flush; the two DMA triggers (~600ns DIRECT2D
    # each) run on different rings so descriptor generation overlaps
    obf = opool.tile([128, 512], F32, tag="obf", name="obf")
    rings = [nc.sync, nc.sync]
    for q in range(2):
        sl = slice(q * 256, (q + 1) * 256)
        nc.scalar.activation(obf[:, sl], pc[(3, 1)][:, sl], AF.Copy, scale=sig_t[:, 3:4])
        rings[q].dma_start(out_d[3, :, 512 + q * 256 : 512 + (q + 1) * 256], obf[:, sl])


_CACHE = None


def _get_compiled():
    global _CACHE
    if _CACHE is None:
        nc = bacc.Bacc(
            "TRN2", target_bir_lowering=False, debug=False, num_devices=B
        )
        x_d = nc.dram_tensor("x", [NCH, 128, PIX], F32, kind="ExternalInput").ap()
        st_d = nc.dram_tensor("style", [128, NCH], F32, kind="ExternalInput").ap()
        wt_d = nc.dram_tensor(
            "wt", [128, NCH, TAPS, COUT], F32, kind="ExternalInput"
        ).ap()
        out_d = nc.dram_tensor("out", [NCH, 128, PIX], F32, kind="ExternalOutput").ap()
        with tile.TileContext(nc) as tc, ExitStack() as ctx:
            _body(ctx, tc, x_d, st_d, wt_d, out_d)
        nc.compile()
        _CACHE = nc
    return _CACHE


def kernel(x, style, weight):
    """x: (8,512,32,32) f32, style: (8,512) f32, weight: (512,512,3,3) f32
    -> (8,512,32,32) f32"""
    global LAST_RESULTS
    x = np.ascontiguousarray(np.asarray(x, dtype=np.float32))
    style = np.asarray(style, dtype=np.float32)
    weight = np.asarray(weight, dtype=np.float32)

    # Host-side layout only (no arithmetic): lhsT weight layout
    # wt[i_lo, c, t, o] = weight[o, c*128 + i_lo, t//3, t%3]
    wt = np.ascontiguousarray(
        weight.reshape(COUT, NCH, 128, TAPS).transpose(2, 1, 3, 0)
    )
    in_maps = []
    for b in range(B):
        in_maps.append(
            {
                "x": x[b].reshape(NCH, 128, PIX),
                "style": np.ascontiguousarray(style[b].reshape(NCH, 128).T),
                "wt": wt,
            }
        )

    nc = _get_compiled()
    res = run_bass_kernel_spmd(nc, in_maps, list(range(B)), trace=TRACE)
    LAST_RESULTS = res
    out = np.empty((B, COUT, H, W), dtype=np.float32)
    for b in range(B):
        out[b] = res.results[b]["out"].reshape(COUT, H, W)
    return out


# revision 44
# speedup vs baseline: 1.0187x; 1.0071x over previous
"""Trainium2 SPMD kernel: StyleGAN2-style modulated conv (Conv2dWeightModulate).

Reference math (per batch sample b):
    w0        = weight * RC                       (equalized-lr scale)
    ws        = w0 * style[b][None,:,None,None]   (per-input-channel modulation)
    sigma_inv = rsqrt(sum_{I,K,K} ws^2 + eps)     (per-output-channel demodulation)
    out[b]    = conv2d(x[b], ws * sigma_inv, pad=1)

Because the modulation is a per-input-channel scale and conv is linear, this
factorizes into ops with a SHARED weight across the batch:
    out[b] = sigma_inv[b,:] * conv2d(x[b] * (style[b]*RC), weight)
    sigma_inv[b,o] = rsqrt(RC^2 * sum_{i,t} weight[o,i,t]^2 * style[b,i]^2 + eps)

Sharding: data-parallel over batch: 8 samples -> 8 NeuronCores, weight
replicated (the groups=b conv factorizes exactly across the batch).

Schedule (v4): all 8 PSUM banks hold the full [512 cout x 1024 px] output at
once as groups (oc, h) = (4 cout chunks x 2 pixel halves); the matmul stream
is chunk-major with all 8 groups interleaved so there is no PSUM wave
barrier. Matmuls run in bf16: the fp32r self-loading matmul's 190ns weight
load limits the cadence to ~245ns, bf16 loads halve that and reach the
213ns roofline. Weights are cast fp32->bf16 on DVE (GpSimd casts run at ~4
cycles/elem -- measured, do not move them back); DVE work is emitted in
consumption order (cast c, scale x_c, sigma-adds c-1) so the in-order queue
never parks behind a late DMA. The last input chunk runs oc/h-major so
groups retire every ~2us: oc=0 flushes UNSCALED to free its banks for the
sigma reduction, later groups flush through ACT with the sigma scale baked
in, and the final group's flush is split in quarters to pipeline ACT with
the output DMA.
"""

from contextlib import ExitStack

import numpy as np

import concourse.bass as bass
import concourse.tile as tile
from concourse import bacc, mybir
from concourse.bass_utils import run_bass_kernel_spmd

B = 8
CIN = 512
COUT = 512
KK = 3
H = 32
W = 32
PIX = H * W
NCH = 4  # channel chunks of 128
TAPS = KK * KK
RC = float(1.0 / np.sqrt(CIN * KK * KK))
EPS = 1e-8
F32 = mybir.dt.float32
F32R = mybir.dt.float32r
BF16 = mybir.dt.bfloat16
AF = mybir.ActivationFunctionType

# test.py toggles these; the grading harness just calls kernel().
TRACE = False
LAST_RESULTS = None


def _body(ctx, tc, x_d, st_d, wt_d, out_d):
    nc = tc.nc
    const = ctx.enter_context(tc.tile_pool(name="const", bufs=1))
    wpool = ctx.enter_context(tc.tile_pool(name="wpool", bufs=1))
    xpool = ctx.enter_context(tc.tile_pool(name="xpool", bufs=1))
    sqpool = ctx.enter_context(tc.tile_pool(name="sqpool", bufs=3))
    opool = ctx.enter_context(tc.tile_pool(name="opool", bufs=4))
    psum = ctx.enter_context(
        tc.tile_pool(name="psum", bufs=1, space=bass.MemorySpace.PSUM)
    )

    # --- all 8 PSUM banks: conv groups (oc chunk, pixel half) ---
    groups = [(oc, h) for oc in range(NCH) for h in range(2)]
    pc = {
        g: psum.tile([128, 512], F32, tag=f"g{g[0]}{g[1]}", name=f"pc{g[0]}{g[1]}")
        for g in groups
    }

    # --- PE pre-warm: ~3us of dummy matmuls so the HAM clock-gate is
    # already released (2.4 GHz) when the first real matmul issues.
    # Warm output aliases bank (0,0); conv's start=True resets it. ---
    # Full-size warms: tiny [1,1]x[1,128] warms leave the clock at the 1.2
    # GHz p-state and the first ~14 real matmuls then run at 427ns
    # (measured); only full-array load ramps the HAM clock to 2.4 GHz.
    # ~24 of these span the input-DMA window (~6.9 -> ~13.8us).
    warm_src = const.tile([128, 512], BF16, tag="warm_src")
    nc.vector.memset(warm_src[:], 0.001)
    warm_w = const.tile([128, 128], BF16, tag="warm_w")
    nc.vector.memset(warm_w[:], 0.001)
    warm_ps = pc[(0, 0)][:, :]
    for _ in range(14):
        nc.tensor.matmul(warm_ps, warm_w[:], warm_src[:], start=True, stop=True)

    # --- x chunk 0 + style are the critical path for the first matmul:
    # trigger them first, split x0 across the sync+gpsimd rings. ---
    # Chunk 0's padded picture is split into two tiles (rows 0-17 / 16-33,
    # two-row overlap re-fetched): dependencies are tile-granular, so with
    # one tile the first matmul waits for the whole of x0 + both scale ops.
    xs = []
    xst = []
    for c in range(NCH):
        if c == 0:
            xa = xpool.tile([128, 18, W + 2], BF16, tag="xs0a", name="xs0a")
            xb = xpool.tile([128, 18, W + 2], BF16, tag="xs0b", name="xs0b")
            xta = xpool.tile([128, 17, W], F32, tag="xst0a", name="xst0a")
            xtb = xpool.tile([128, 17, W], F32, tag="xst0b", name="xst0b")
            xs.append((xa, xb))
            xst.append((xta, xtb))
        else:
            xc = xpool.tile([128, H + 2, W + 2], BF16, tag=f"xs{c}", name=f"xs{c}")
            xt = xpool.tile([128, H, W], F32, tag=f"xst{c}", name=f"xst{c}")
            xs.append(xc)
            xst.append(xt)

    # style goes first: its 128 16-byte descriptors complete ~0.6us after
    # entering the queues, but queued behind x0 they'd finish ~3us late and
    # st_rc gates the x-scale on the first-matmul critical path.
    st = const.tile([128, NCH], F32, tag="st")
    x0 = x_d[0].rearrange("p (h w) -> p h w", h=H)
    nc.sync.dma_start(st[:], st_d[:])
    nc.sync.dma_start(xst[0][0][:], x0[:, 0:17, :])

    st_rc = const.tile([128, NCH], F32, tag="st_rc")
    nc.vector.tensor_scalar_mul(st_rc[:], st[:], RC)
    st2 = const.tile([128, NCH], BF16, tag="st2")
    nc.vector.tensor_mul(st2[:], st[:], st[:])

    # padded-picture borders on GpSimd (it has nothing else to do)
    xa, xb = xs[0]
    nc.gpsimd.memset(xa[:, 0, :], 0.0)
    nc.gpsimd.memset(xa[:, 1:18, 0], 0.0)
    nc.gpsimd.memset(xa[:, 1:18, W + 1], 0.0)
    nc.gpsimd.memset(xb[:, 17, :], 0.0)
    nc.gpsimd.memset(xb[:, 0:17, 0], 0.0)
    nc.gpsimd.memset(xb[:, 0:17, W + 1], 0.0)
    for c in range(1, NCH):
        nc.gpsimd.memset(xs[c][:, 0, :], 0.0)
        nc.gpsimd.memset(xs[c][:, H + 1, :], 0.0)
        nc.gpsimd.memset(xs[c][:, 1 : H + 1, 0], 0.0)
        nc.gpsimd.memset(xs[c][:, 1 : H + 1, W + 1], 0.0)

    # --- weights: per-tap triggers for chunk 0 so tap t streams just ahead
    # of the PE; later chunks one trigger each; x1-3 behind the weight
    # chunk that precedes their use. All on the scalar ring. ---
    wt = [
        wpool.tile([128, TAPS, COUT], F32, tag=f"wt{c}", name=f"wt{c}")
        for c in range(NCH)
    ]
    wt16 = [
        wpool.tile([128, TAPS, COUT], BF16, tag=f"wt16_{c}", name=f"wt16_{c}")
        for c in range(NCH)
    ]
    nc.gpsimd.dma_start(xst[0][1][:], x0[:, 15:H, :])
    for t in range(TAPS):
        nc.scalar.dma_start(wt[0][:, t], wt_d[:, 0, t])

    # Per-chunk sum over taps of squared weights (ACT squares, DVE adds):
    # cuts the PE cost of the sigma reduction from 36 matmuls to 4.
    w2s = {}

    def sig_squares(c):
        parts = []
        for t in range(TAPS):
            w2 = sqpool.tile([128, COUT], BF16, tag=f"w2_{t % 3}", name="w2")
            nc.scalar.activation(w2[:], wt[c][:, t], AF.Square)
            parts.append(w2)
            if t == 1:
                acc = sqpool.tile([128, COUT], BF16, tag=f"w2s{c}", name="w2s")
                nc.vector.tensor_add(acc[:], parts[0][:], parts[1][:])
            elif t > 1:
                nc.vector.tensor_add(acc[:], acc[:], parts[-1][:])
        w2s[c] = acc

    # DVE in consumption order: cast chunk c, scale x_c, sigma-adds c-1.
    # ACT squares are emitted here too (in-order per engine, c-major).
    def casts(c, lo=0, hi=TAPS):
        for t in range(lo, hi):
            nc.vector.tensor_copy(wt16[c][:, t], wt[c][:, t])

    def xscale(c):
        nc.vector.tensor_scalar_mul(
            xs[c][:, 1 : H + 1, 1 : W + 1], xst[c][:], st_rc[:, c : c + 1]
        )

    # head: tap 0 is cast on ACT (in parallel with DVE's x-scale; both are
    # on the first-matmul critical path), DVE scales the rows the h=0
    # groups touch (arriving on the sync ring alone), then backfills.
    nc.scalar.activation(wt16[0][:, 0], wt[0][:, 0], AF.Copy)
    nc.vector.tensor_scalar_mul(xa[:, 1:18, 1 : W + 1], xst[0][0][:], st_rc[:, 0:1])
    nc.vector.tensor_scalar_mul(xb[:, 0:17, 1 : W + 1], xst[0][1][:], st_rc[:, 0:1])
    casts(0, 1, 3)
    # Gate the chunk-1..3 DMAs (8.1 MB) behind the chunk-0 tap-2 cast: a
    # 1-element DVE copy reading wt16[0] tap 2 (a real dependency the
    # scheduler can't hoist) writes each gated tile, and the dma_start
    # emitted after it picks up the WAW dep. Keeps 8.1 MB of descriptors
    # out of the 16 DMA queues while x0 and the chunk-0 taps -- the
    # first-matmul critical path -- are in flight.
    for c in range(1, NCH):
        nc.vector.tensor_copy(wt[c][0:1, 0:1, 0:1], wt16[0][0:1, 2:3, 0:1])
        nc.scalar.dma_start(wt[c][:], wt_d[:, c])
        nc.vector.tensor_copy(xst[c][0:1, 0:1, 0:1], wt16[0][0:1, 2:3, 0:1])
        nc.scalar.dma_start(xst[c][:], x_d[c].rearrange("p (h w) -> p h w", h=H))
    casts(0, 3, TAPS)
    sig_squares(0)
    casts(1)
    xscale(1)
    sig_squares(1)
    casts(2)
    xscale(2)
    sig_squares(2)
    casts(3)
    xscale(3)
    sig_squares(3)

    eps_b = const.tile([1, 1], F32, tag="eps_b")
    nc.gpsimd.memset(eps_b[:], EPS)
    ones_t = const.tile([1, 1], F32, tag="ones_t")
    nc.gpsimd.memset(ones_t[:], 1.0)
    sig_sq = const.tile([1, COUT], F32, tag="sig_sq")
    sig_sd = const.tile([128, NCH], F32, tag="sig_sd")
    sig_t = const.tile([128, NCH], F32, tag="sig_t")

    # preload the Sqrt activation table while ACT is otherwise idle
    nc.scalar.activation(sig_sq[0:1, 0:1], eps_b[:], AF.Sqrt)

    # sigma PSUM space aliases group oc=0's banks, free after its early flush
    sig_ps = pc[(0, 0)][0:1, :]
    sig_tp = pc[(0, 1)][:, 0:NCH]

    out_ring = {0: nc.gpsimd, 1: nc.sync, 2: nc.gpsimd, 3: nc.sync}

    def flush_scaled(g):
        oc, h = g
        ob = opool.tile([128, 512], F32, tag=f"ob{h}", name=f"ob{h}")
        nc.scalar.activation(ob[:], pc[g][:], AF.Copy, scale=sig_t[:, oc : oc + 1])
        out_ring[oc].dma_start(out_d[oc, :, h * 512 : (h + 1) * 512], ob[:])

    def conv_mm(g, c, t, start, stop):
        oc, h = g
        dy, dx = t // 3, t % 3
        if c == 0:
            rhs = xs[0][h][:, dy : dy + 16, dx : dx + W]
        else:
            rhs = xs[c][:, dy + h * 16 : dy + h * 16 + 16, dx : dx + W]
        nc.tensor.matmul(
            pc[g][:],
            wt16[c][:, t, oc * 128 : (oc + 1) * 128],
            rhs,
            start=start,
            stop=stop,
        )

    # --- conv stream: chunks 0-2 with oc innermost (all 8 groups in
    # flight; h=0 groups first so chunk 0 starts before the second half of
    # x0 is scaled); chunk 3 oc/h-major so groups retire every ~2us. ---
    h_major = [(oc, h) for h in range(2) for oc in range(NCH)]
    for c in range(3):
        first = c == 0
        for t in range(TAPS):
            for g in h_major:
                conv_mm(g, c, t, first and t == 0, False)

    ou0 = [
        opool.tile([128, 512], F32, tag=f"ou0{h}", name=f"ou0{h}") for h in range(2)
    ]

    def sig_mms():
        for c in range(NCH):
            nc.tensor.matmul(
                sig_ps,
                st2[:, c : c + 1],
                w2s[c][:],
                start=(c == 0),
                stop=(c == NCH - 1),
            )

    def sig_transposes():
        # sqrt result [1,512] -> PE-transpose -> [128,4]
        for oc in range(NCH):
            nc.tensor.transpose(
                sig_tp[:, oc : oc + 1],
                sig_sq[0:1, oc * 128 : (oc + 1) * 128],
                ones_t[:],
            )

    # chunk 3, group-retiring order with the sigma pipeline woven in:
    #   PE : [oc0 h0] [oc0 h1] [sig mms] [oc1 h0] [transposes] [oc1 h1] ...
    #   ACT: ........ [flush oc0h0 raw] [flush oc0h1 raw] [sqrt] [flush oc1 scaled] ...
    #   DVE: ................................... [copy+recip] ... [scale ou0] ...
    for oc in range(NCH):
        for h in range(2):
            for t in range(TAPS):
                conv_mm((oc, h), 3, t, False, t == TAPS - 1)
            if oc == 0:
                # unscaled flush frees bank (0,h) for the sigma reduction
                nc.scalar.activation(ou0[h][:], pc[(0, h)][:], AF.Copy)
                if h == 1:
                    sig_mms()
            elif oc == 1:
                if h == 0:
                    nc.scalar.activation(
                        sig_sq[:], sig_ps, AF.Sqrt, bias=eps_b[:], scale=RC * RC
                    )
                    sig_transposes()
                else:
                    nc.vector.tensor_copy(sig_sd[:], sig_tp[:, 0:NCH])
                    nc.vector.reciprocal(sig_t[:], sig_sd[:])
                    flush_scaled((1, 0))
            elif oc == 2:
                if h == 0:
                    flush_scaled((1, 1))
                    # group 0 was flushed unscaled; apply sigma on DVE
                    for hh in range(2):
                        nc.vector.tensor_scalar_mul(
                            ou0[hh][:], ou0[hh][:], sig_t[:, 0:1]
                        )
                        out_ring[2 * hh].dma_start(
                            out_d[0, :, hh * 512 : (hh + 1) * 512], ou0[hh][:]
                        )
                else:
                    flush_scaled((2, 0))
            else:
                if h == 0:
                    flush_scaled((2, 1))
                    flush_scaled((3, 0))

    # final group: half-split flush pipelines ACT with the out DMA
    obf = opool.tile([128, 512], F32, tag="obf", name="obf")
    for q in range(2):
        sl = slice(q * 256, (q + 1) * 256)
        nc.scalar.activation(
            obf[:, sl], pc[(3, 1)][:, sl], AF.Copy, scale=sig_t[:, 3:4]
        )
        nc.sync.dma_start(out_d[3, :, 512 + q * 256 : 512 + (q + 1) * 256], obf[:, sl])


_CACHE = None


def _get_compiled():
    global _CACHE
    if _CACHE is None:
        nc = bacc.Bacc(
            "TRN2", target_bir_lowering=False, debug=False, num_devices=B
        )
        x_d = nc.dram_tensor("x", [NCH, 128, PIX], F32, kind="ExternalInput").ap()
        st_d = nc.dram_tensor("style", [128, NCH], F32, kind="ExternalInput").ap()
        wt_d = nc.dram_tensor(
            "wt", [128, NCH, TAPS, COUT], F32, kind="ExternalInput"
        ).ap()
        out_d = nc.dram_tensor("out", [NCH, 128, PIX], F32, kind="ExternalOutput").ap()
        with tile.TileContext(nc) as tc, ExitStack() as ctx:
            _body(ctx, tc, x_d, st_d, wt_d, out_d)
        nc.compile()
        _CACHE = nc
    return _CACHE


def kernel(x, style, weight):
    """x: (8,512,32,32) f32, style: (8,512) f32, weight: (512,512,3,3) f32
    -> (8,512,32,32) f32"""
    global LAST_RESULTS
    x = np.ascontiguousarray(np.asarray(x, dtype=np.float32))
    style = np.asarray(style, dtype=np.float32)
    weight = np.asarray(weight, dtype=np.float32)

    # Host-side layout only (no arithmetic): lhsT weight layout
    # wt[i_lo, c, t, o] = weight[o, c*128 + i_lo, t//3, t%3]
    wt = np.ascontiguousarray(
        weight.reshape(COUT, NCH, 128, TAPS).transpose(2, 1, 3, 0)
    )
    in_maps = []
    for b in range(B):
        in_maps.append(
            {
                "x": x[b].reshape(NCH, 128, PIX),
                "style": np.ascontiguousarray(style[b].reshape(NCH, 128).T),
                "wt": wt,
            }
        )

    nc = _get_compiled()
    res = run_bass_kernel_spmd(nc, in_maps, list(range(B)), trace=TRACE)
    LAST_RESULTS = res
    out = np.empty((B, COUT, H, W), dtype=np.float32)
    for b in range(B):
        out[b] = res.results[b]["out"].reshape(COUT, H, W)
    return out
